# revision 7
# baseline (speedup 1.0000x reference)
"""CenterLoss kernel for Trainium2 (Bass/Tile), 8-core SPMD.

Math: the reference computes
    distmat = ||x||^2 + ||c||^2 - 2 x@c^T        [B, C]
    loss = sum(clip(distmat * onehot(labels), 1e-12, 1e12)) / B
Only the B label-gathered entries of distmat survive the mask; every other
element is clipped from 0 up to exactly 1e-12.  So
    loss = ( sum_i clip(||x_i - centers[labels_i]||^2, 1e-12, 1e12)
             + B*(C-1)*1e-12 ) / B
No BxC distmat is needed.

Sharding: BATCH-sharded.  Core k owns x rows [64k, 64k+64) (a contiguous
shard, direct DMA - no gather and no index dependency) and receives the
full centers table plus its 64 labels as int32 row offsets.  Every core
always owns exactly 64 rows regardless of the label distribution, so there
is no capacity fallback path at all.

Per-core program: a tiny DMA stages the 64 int32 offsets into SBUF, ONE
indirect DMA gathers the 64 label rows of centers, and the x-shard load
(direct HWDGE DMA) overlaps with the gather.  ||x_i - c_i||^2 is computed
in column halves (DVE subtract pipelined with ACT square+row-accumulate)
and the per-row partial sums are DMA'd out; the host folds halves, applies
the clip, adds the closed-form masked-zeros constant, and divides by B.

(A variant whose gather read its offsets directly from DRAM - skipping the
SBUF staging DMA - simulates ~2us faster but is rejected by walrus codegen
at generateDynamicDMA, so the SBUF-staged shape is the only variant.)

The Bass builders are exec'd from a source string compiled under a fixed
pseudo-filename so the emitted BIR (which embeds builder file/line debug
info) is byte-identical regardless of where this file lives - keeping the
NEFF compile cache warm across directories.
"""

import numpy as np

B, D, C = 512, 1024, 50000
N_CORES = 8
R = B // N_CORES  # x rows per core (batch shard)
CLAMP_MIN = 1e-12
CLAMP_MAX = 1e12

_NC_CACHE = {}

_BUILDER_SRC = '''
B, D, C = 512, 1024, 50000
N_CORES = 8
R = B // N_CORES
SPLIT = 2  # column halves pipelined across DVE/ACT; folded on the host


def _new_nc():
    import concourse.bacc as bacc

    return bacc.Bacc(
        "TRN2",
        target_bir_lowering=False,
        debug=False,
        num_devices=N_CORES,
        num_swdge_queues=2,
    )


def build(idx_via_sbuf):
    import concourse.bass as bass
    import concourse.mybir as mybir
    import concourse.tile as tile

    nc = _new_nc()
    x_d = nc.dram_tensor("xshard", [R, D], mybir.dt.float32, kind="ExternalInput")
    c_d = nc.dram_tensor("centers", [C, D], mybir.dt.float32, kind="ExternalInput")
    i_d = nc.dram_tensor("cidx", [R, 1], mybir.dt.int32, kind="ExternalInput")
    o_d = nc.dram_tensor("partial", [R, SPLIT], mybir.dt.float32, kind="ExternalOutput")

    # asymmetric halves: a short first slice gets DVE's subtract (and so
    # ACT's square+accumulate) started sooner; ACT then streams the longer
    # tail slice while DVE finishes it in parallel.
    BOUNDS = [0, 384, D]
    with tile.TileContext(nc) as tc:
        with tc.tile_pool(name="sbuf", bufs=1) as pool:
            g_sb = pool.tile([R, D], mybir.dt.float32)
            if idx_via_sbuf:
                idx_sb = pool.tile([R, 1], mybir.dt.int32)
                nc.sync.dma_start(idx_sb[:], i_d[:])
                off_ap = idx_sb[:, :1]
            else:
                # offsets read straight from DRAM by descriptor generation
                off_ap = i_d[:, :1]
            nc.gpsimd.indirect_dma_start(
                out=g_sb[:],
                out_offset=None,
                in_=c_d[:, :],
                in_offset=bass.IndirectOffsetOnAxis(ap=off_ap, axis=0),
            )
            x_sb = pool.tile([R, D], mybir.dt.float32)
            nc.sync.dma_start(x_sb[:], x_d[:])

            diff = pool.tile([R, D], mybir.dt.float32)
            sq = pool.tile([R, D], mybir.dt.float32)
            rs = pool.tile([R, SPLIT], mybir.dt.float32)
            for h in range(SPLIT):
                sl = slice(BOUNDS[h], BOUNDS[h + 1])
                nc.vector.tensor_tensor(
                    out=diff[:, sl], in0=x_sb[:, sl], in1=g_sb[:, sl],
                    op=mybir.AluOpType.subtract,
                )
                # ACT squares AND row-reduces via its accumulator, so the
                # DVE only does the subtracts
                nc.scalar.activation(
                    sq[:, sl], diff[:, sl], mybir.ActivationFunctionType.Square,
                    accum_out=rs[:, h : h + 1],
                )
            nc.sync.dma_start(o_d[:], rs[:])

    nc.compile()
    return nc
'''

_builder_ns = {}
exec(compile(_BUILDER_SRC, "<centerloss_kernel>", "exec"), _builder_ns)
SPLIT = _builder_ns["SPLIT"]


def _get_nc(which="sbuf_idx"):
    if which not in _NC_CACHE:
        _NC_CACHE[which] = _builder_ns["build"](which == "sbuf_idx")
    return _NC_CACHE[which]


def _make_in_maps(x, labels_i, centers):
    in_maps = []
    for k in range(N_CORES):
        sl = slice(k * R, (k + 1) * R)
        in_maps.append(
            {
                "xshard": x[sl],
                "centers": centers,
                "cidx": np.ascontiguousarray(
                    labels_i[sl].astype(np.int32).reshape(R, 1)
                ),
            }
        )
    return in_maps


def _loss_from_d(d):
    d = np.clip(d.astype(np.float64), CLAMP_MIN, CLAMP_MAX)
    loss = (d.sum() + B * (C - 1) * CLAMP_MIN) / B
    return np.array(loss, dtype=np.float32)


def _poke_devices():
    """Nudge the accelerators with a trivial jitted op to clear wedges."""
    try:
        import jax
        import jax.numpy as jnp

        a = jnp.ones((64, 64), dtype=jnp.float32)
        jax.jit(jnp.dot)(a, a).block_until_ready()
    except Exception:
        pass


def _reset_backend():
    """Drop the PJRT client so the next use opens a fresh device session."""
    try:
        import jax

        clear = getattr(
            getattr(getattr(jax, "extend", None), "backend", None),
            "clear_backends",
            None,
        ) or getattr(jax, "clear_backends", None)
        if clear is not None:
            clear()
    except Exception:
        pass


# NRT_EXEC_UNIT_UNRECOVERABLE wedges on the shared terminal have been seen
# to heal only after ~1-3 minutes, so back off patiently before giving up.
_RETRY_SLEEPS = (5.0, 10.0, 20.0, 40.0, 60.0)


def _run_spmd(nc, in_maps, **kwargs):
    """run_bass_kernel_spmd with retries for transient device wedges."""
    import time as _time

    from concourse.bass_utils import run_bass_kernel_spmd

    last = None
    for attempt in range(len(_RETRY_SLEEPS) + 1):
        try:
            return run_bass_kernel_spmd(
                nc, in_maps, core_ids=list(range(N_CORES)), **kwargs
            )
        except Exception as e:  # transient NRT/axon wedges heal on retry
            last = e
            if attempt >= len(_RETRY_SLEEPS):
                break
            _time.sleep(_RETRY_SLEEPS[attempt])
            _reset_backend()
            _poke_devices()
    raise last


def _spot_check(d, x, labels_i, centers):
    """Verify a few rows against host math; flags silent device corruption.

    A wedged NeuronCore has been observed to return garbage without raising,
    and the DRAM-offset gather variant is validated end-to-end by this same
    check.  Recomputing ||x_i - c_{label_i}||^2 for 8 of 512 rows costs ~25k
    flops on the host and catches both cases so the caller can retry or fall
    back.
    """
    rows = np.linspace(0, B - 1, 8).astype(np.int64)
    xs = x[rows].astype(np.float64)
    cs = centers[labels_i[rows]].astype(np.float64)
    want = ((xs - cs) ** 2).sum(axis=1)
    rel = np.abs(d[rows] - want) / np.maximum(np.abs(want), 1e-9)
    return bool((rel < 1e-3).all())


def _device_d(which, in_maps):
    nc = _get_nc(which)
    res = _run_spmd(nc, in_maps)
    # fold the SPLIT per-column-half partial sums on the host
    return np.concatenate(
        [res.results[k]["partial"].astype(np.float64).sum(axis=1) for k in range(N_CORES)]
    )


def kernel(x, labels, centers):
    x = np.ascontiguousarray(np.asarray(x, dtype=np.float32))
    centers = np.ascontiguousarray(np.asarray(centers, dtype=np.float32))
    labels_i = np.asarray(labels).astype(np.int64).reshape(B)
    in_maps = _make_in_maps(x, labels_i, centers)

    for attempt in range(4):
        d = _device_d("sbuf_idx", in_maps)
        if _spot_check(d, x, labels_i, centers):
            return _loss_from_d(d)
        import time as _time

        _time.sleep(3.0 * (attempt + 1))
        _poke_devices()
    raise RuntimeError(
        "device results failed host spot-check repeatedly (wedged NeuronCores?)"
    )
